# revision 6
# baseline (speedup 1.0000x reference)
"""Trainium2 Bass kernel for the CustomCheckMessageGNNLayer min-sum check update.

Problem structure (hardcoded, per the problem spec):
  message_features: (B=4, M=393216, H=64) f32
  check_index_tensor = arange(C*D).reshape(C=49152, D=8)  -> identity gather/scatter,
  mask all-true, deg=8 everywhere; message_types unused by the reference.

Computation:
  llr[b,m]   = dot(message_features[b,m,:], proj_w) + proj_b
  per check c (messages 8c..8c+7): leave-one-out min-sum:
      vals[b,c,j] = alpha * (prod_i sign(llr_i)) * sign(llr_j) * loo_min_j
      loo_min_j   = min2 if |llr_j| == min1 else min1   (min1/min2 = order stats)
  output = message_features with channel 0 replaced by scattered vals.

Sharding: checks are split across the 8 cores; batch stays interleaved into the
per-core check-instance stream (the min-sum is purely per-check, so batch
boundaries are irrelevant on device). alpha (>0) is folded into proj_w on the
host: scaling all llrs by alpha>0 commutes with sign/min order statistics and
scales the output linearly.

Device pipeline (per core):
  - Input staged host-side as f16 in a PE-friendly layout: per PSUM bank-group
    of 65536 messages, partition p holds feature-a slabs of its 512 messages
    (free = a*512 + j*64 + t, j-major message order).
  - The H-dot runs on the (otherwise idle) TensorEngine as 64 accumulating
    matmuls per bank-group: lhsT_a = diag(alpha*w[a]) (128x128 scaled identity,
    host-staged), rhs = feature-a slab (128x512). PSUM accumulates llrs in f32,
    landing dense (128, 512) j-major.
  - Min-sum reads PSUM directly (no evacuation copy): |.| on ACT, signs and
    sign-product tournament on Pool, min1/min2 tournament + leave-one-out
    select + final products on DVE, broadcast materialization on ACT.
  - Only the llr plane (vals) is written back; the host assembles the full
    output (copy of untouched input channels + channel-0 scatter), pure data
    movement.
"""

import os
import sys
from contextlib import ExitStack

import numpy as np

for _p in ("/opt/trn_rl_repo", "/opt/trn_rl_repo/concourse"):
    if _p not in sys.path and os.path.isdir(_p):
        sys.path.insert(0, _p)

# ---- problem geometry (fixed by the spec) ----
B, M, H = 4, 393216, 64
C, D = 49152, 8
NCORES = 8
CS = C // NCORES          # 6144 checks per core
CI = B * CS               # 24576 check-instances per core (batch-major)
PT = 128                  # partitions
NG = 3                    # PSUM bank-groups per core
NT = 64                   # checks per partition per group
GW = D * NT               # 512 llrs per partition per group (j-major)
NSUB = 8                  # DMA sub-tiles per group (8 feature-slabs each)
SUBW = 8 * GW             # 4096 elements per sub-tile per partition

_CACHE: dict = {}

# test-harness hooks: extra kwargs for run_bass_kernel_spmd (e.g. tracing) and
# the last BassKernelResults for reading exec_time_ns. Unused when grading.
RUN_KW: dict = {}
last_results = None


def _build(bias: float):
    """Trace + compile the per-core Bass kernel.

    Inputs:
      x: (NG, PT, H*GW) f16 -- per-core message features, bank-group-major,
         free = a*GW + j*NT + t  (feature-slab-major, j-major messages)
      w: (PT, H*PT) f16     -- 64 concatenated 128x128 scaled identities,
         w[p, a*PT + p] = alpha*proj_w[a]
    Output:
      o: (NG, PT, GW) f32   -- min-sum vals, same j-major layout as x slabs
    """
    import concourse.bass as bass  # noqa: F401
    import concourse.tile as tile
    from concourse import bacc, mybir

    f32 = mybir.dt.float32
    f16 = mybir.dt.bfloat16
    op = mybir.AluOpType

    nc = bacc.Bacc(
        "TRN2",
        target_bir_lowering=False,
        debug=False,
        enable_asserts=False,
        num_devices=NCORES,
    )
    x_d = nc.dram_tensor("x", [NG, PT, NSUB * SUBW], f16, kind="ExternalInput").ap()
    w_d = nc.dram_tensor("w", [PT, H * PT], f16, kind="ExternalInput").ap()
    o_d = nc.dram_tensor("o", [NG, PT, GW], f32, kind="ExternalOutput").ap()

    with tile.TileContext(nc) as tc, ExitStack() as ctx:
        wpool = ctx.enter_context(tc.tile_pool(name="wid", bufs=1))
        xpool = ctx.enter_context(tc.tile_pool(name="x", bufs=6))
        pspool = ctx.enter_context(tc.tile_pool(name="ps", bufs=3, space="PSUM"))
        mpool = ctx.enter_context(tc.tile_pool(name="ms", bufs=2))

        # identity-weight tile arrives in 8 chunks interleaved with group-0
        # x sub-tiles so the first matmul doesn't wait on the full 2MB
        w_t = wpool.tile([PT, H * PT], f16)
        WCH = H * PT // NSUB
        for g in range(NG):
            ps = pspool.tile([PT, GW], f32, tag="ps")
            for s in range(NSUB):
                if g == 0:
                    nc.sync.dma_start(
                        w_t[:, s * WCH : (s + 1) * WCH],
                        w_d[:, s * WCH : (s + 1) * WCH],
                    )
                xt = xpool.tile([PT, SUBW], f16, tag="xt")
                nc.sync.dma_start(xt[:], x_d[g, :, s * SUBW : (s + 1) * SUBW])
                for al in range(8):
                    a = s * 8 + al
                    nc.tensor.matmul(
                        ps[:],
                        w_t[:, a * PT : (a + 1) * PT],
                        xt[:, al * GW : (al + 1) * GW],
                        start=(s == 0 and al == 0),
                        stop=(s == NSUB - 1 and al == 7),
                    )

            # ---- leave-one-out min-sum on ps (PT, GW), j-major, nt=NT ----
            # |g| on ACT straight out of PSUM; sign as 2*(g>=0)-1 in {-1,+1}
            # on Pool. Never-zero sign keeps the leave-one-out sign product
            # correct even if an llr rounds to exactly 0.
            g_src = ps[:]
            if bias != 0.0:
                gb = mpool.tile([PT, GW], f32, tag="gb")
                nc.gpsimd.tensor_scalar_add(gb[:], ps[:], bias)
                g_src = gb[:]
            a_t = mpool.tile([PT, GW], f32, tag="abs")
            nc.scalar.activation(a_t[:], g_src, mybir.ActivationFunctionType.Abs)
            sge = mpool.tile([PT, GW], f32, tag="sge")
            s_t = mpool.tile([PT, GW], f32, tag="sgn")
            nc.vector.tensor_scalar(sge[:], g_src, 0.0, None, op0=op.is_ge)
            nc.gpsimd.tensor_scalar(s_t[:], sge[:], 2.0, -1.0, op0=op.mult,
                                    op1=op.add)

            q = GW // 2
            # min/max tournament for min1/min2 (exact 2nd order statistic)
            lo1 = mpool.tile([PT, q], f32, tag="lo1")
            hi1 = mpool.tile([PT, q], f32, tag="hi1")
            nc.vector.tensor_tensor(lo1[:], a_t[:, 0:q], a_t[:, q:GW], op=op.min)
            nc.vector.tensor_tensor(hi1[:], a_t[:, 0:q], a_t[:, q:GW], op=op.max)

            m1_2 = mpool.tile([PT, q // 2], f32, tag="m1_2")
            x2 = mpool.tile([PT, q // 2], f32, tag="x2")
            y2 = mpool.tile([PT, q // 2], f32, tag="y2")
            m2_2 = mpool.tile([PT, q // 2], f32, tag="m2_2")
            nc.vector.tensor_tensor(m1_2[:], lo1[:, 0 : q // 2], lo1[:, q // 2 : q], op=op.min)
            nc.vector.tensor_tensor(x2[:], lo1[:, 0 : q // 2], lo1[:, q // 2 : q], op=op.max)
            nc.vector.tensor_tensor(y2[:], hi1[:, 0 : q // 2], hi1[:, q // 2 : q], op=op.min)
            nc.vector.tensor_tensor(m2_2[:], x2[:], y2[:], op=op.min)

            min1 = mpool.tile([PT, NT], f32, tag="min1")
            x3 = mpool.tile([PT, NT], f32, tag="x3")
            y3 = mpool.tile([PT, NT], f32, tag="y3")
            min2 = mpool.tile([PT, NT], f32, tag="min2")
            nc.vector.tensor_tensor(min1[:], m1_2[:, 0:NT], m1_2[:, NT : 2 * NT], op=op.min)
            nc.vector.tensor_tensor(x3[:], m1_2[:, 0:NT], m1_2[:, NT : 2 * NT], op=op.max)
            nc.vector.tensor_tensor(y3[:], m2_2[:, 0:NT], m2_2[:, NT : 2 * NT], op=op.min)
            nc.vector.tensor_tensor(min2[:], x3[:], y3[:], op=op.min)

            # sign product per check (tournament of multiplies) on Pool;
            # runs concurrently with the DVE min tournament above
            s1 = mpool.tile([PT, q], f32, tag="s1")
            nc.gpsimd.tensor_tensor(s1[:], s_t[:, 0:q], s_t[:, q:GW], op=op.mult)
            s2 = mpool.tile([PT, q // 2], f32, tag="s2")
            nc.gpsimd.tensor_tensor(s2[:], s1[:, 0 : q // 2], s1[:, q // 2 : q], op=op.mult)
            ts = mpool.tile([PT, NT], f32, tag="ts")
            nc.gpsimd.tensor_tensor(ts[:], s2[:, 0:NT], s2[:, NT : 2 * NT], op=op.mult)

            # leave-one-out sign sl = s_t * bcast(ts): off the critical path
            # (ready before the min tournament finishes)
            ts_b = ts[:].unsqueeze(1).broadcast_to([PT, D, NT])
            tsf = mpool.tile([PT, GW], f32, tag="tsf")
            nc.scalar.copy(tsf[:].rearrange("p (j t) -> p j t", t=NT), ts_b)
            sl = mpool.tile([PT, GW], f32, tag="sl")
            nc.gpsimd.tensor_tensor(sl[:], s_t[:], tsf[:], op=op.mult)

            # materialize min1/min2 broadcasts on ACT and Pool in parallel
            min1_b = min1[:].unsqueeze(1).broadcast_to([PT, D, NT])
            min2_b = min2[:].unsqueeze(1).broadcast_to([PT, D, NT])
            loo = mpool.tile([PT, GW], f32, tag="loo")
            m2f = mpool.tile([PT, GW], f32, tag="m2f")
            nc.scalar.copy(loo[:].rearrange("p (j t) -> p j t", t=NT), min1_b)
            nc.gpsimd.tensor_copy(m2f[:].rearrange("p (j t) -> p j t", t=NT), min2_b)

            # loo_min = where(|g| == min1, min2, min1), then one fused final
            # product vals = loo_min * sl   (alpha already folded into w)
            msk = mpool.tile([PT, GW], mybir.dt.uint8, tag="msk")
            nc.vector.tensor_tensor(msk[:], a_t[:], loo[:], op=op.is_equal)
            nc.vector.copy_predicated(loo[:], msk[:], m2f[:])
            v2_t = mpool.tile([PT, GW], f32, tag="v2")
            nc.vector.tensor_tensor(v2_t[:], loo[:], sl[:], op=op.mult)
            nc.sync.dma_start(o_d[g], v2_t[:])

    nc.compile()
    return nc


def _get_compiled(bias: float):
    key = (bias,)
    if key not in _CACHE:
        _CACHE[key] = _build(bias)
    return _CACHE[key]


def _prepare(message_features, proj_w, proj_b, alpha):
    """Shard/stage host-side: returns (mf, in_maps, bias)."""
    mf = np.ascontiguousarray(np.asarray(message_features, dtype=np.float32))
    w = np.asarray(proj_w, dtype=np.float32).reshape(H)
    al = float(np.asarray(alpha))
    pb = float(np.asarray(proj_b))
    assert al > 0.0, "kernel assumes alpha > 0 (scaling folded into proj_w)"

    import ml_dtypes
    bf16 = ml_dtypes.bfloat16
    wt = (w * al).astype(bf16)
    wid = np.zeros((PT, H, PT), dtype=bf16)
    wid[np.arange(PT)[:, None], :, np.arange(PT)[:, None]] = wt[None, :]
    wid = wid.reshape(PT, H * PT)
    bias = al * pb

    # per-core staging: check-instance ci = g*8192 + p*64 + t = b*6144 + c
    xr = mf.reshape(B, NCORES, CS * D * H)
    in_maps = []
    for k in range(NCORES):
        xg = xr[:, k].reshape(NG, PT, NT, D, H)         # (g, p, t, j, h)
        xk = xg.transpose(0, 1, 4, 3, 2)                # (g, p, h, j, t)
        xk = np.ascontiguousarray(xk.astype(bf16)).reshape(NG, PT, NSUB * SUBW)
        in_maps.append({"x": xk, "w": wid})
    return mf, in_maps, bias


def _assemble(mf, outs):
    """outs: per-core 'o' arrays (NG, PT, GW) f32 in j-major layout."""
    llr = np.stack(outs)                                   # (K, NG, PT, D*NT)
    llr = llr.reshape(NCORES, NG, PT, D, NT)
    llr = llr.transpose(0, 1, 2, 4, 3).reshape(NCORES, B, CS * D)  # (k, ci, j)
    llr = llr.transpose(1, 0, 2).reshape(B, M)
    out = mf.copy()
    out[:, :, 0] = llr
    return out


def kernel(
    message_features: np.ndarray,
    message_types: np.ndarray,
    check_index_tensor: np.ndarray,
    proj_w: np.ndarray,
    proj_b: np.ndarray,
    alpha: np.ndarray,
) -> np.ndarray:
    from concourse.bass_utils import run_bass_kernel_spmd

    mf, in_maps, bias = _prepare(message_features, proj_w, proj_b, alpha)
    nc = _get_compiled(bias)
    res = run_bass_kernel_spmd(nc, in_maps, core_ids=list(range(NCORES)), **RUN_KW)
    global last_results
    last_results = res
    return _assemble(mf, [r["o"] for r in res.results])


# revision 8
# speedup vs baseline: 1.2164x; 1.2164x over previous
"""Trainium2 Bass kernel for the CustomCheckMessageGNNLayer min-sum check update.

Problem structure (hardcoded, per the problem spec):
  message_features: (B=4, M=393216, H=64) f32
  check_index_tensor = arange(C*D).reshape(C=49152, D=8)  -> identity gather/scatter,
  mask all-true, deg=8 everywhere; message_types unused by the reference.

Computation:
  llr[b,m]   = dot(message_features[b,m,:], proj_w) + proj_b
  per check c (messages 8c..8c+7): leave-one-out min-sum:
      vals[b,c,j] = alpha * (prod_i sign(llr_i)) * sign(llr_j) * loo_min_j
      loo_min_j   = min2 if |llr_j| == min1 else min1   (min1/min2 = order stats)
  output = message_features with channel 0 replaced by scattered vals.

Sharding: checks are split across the 8 cores; batch stays interleaved into the
per-core check-instance stream (the min-sum is purely per-check, so batch
boundaries are irrelevant on device). alpha (>0) is folded into proj_w on the
host: scaling all llrs by alpha>0 commutes with sign/min order statistics and
scales the output linearly.

Device pipeline (per core):
  - Input staged host-side as f16 in a PE-friendly layout: per PSUM bank-group
    of 65536 messages, partition p holds feature-a slabs of its 512 messages
    (free = a*512 + j*64 + t, j-major message order).
  - The H-dot runs on the (otherwise idle) TensorEngine as 64 accumulating
    matmuls per bank-group: lhsT_a = diag(alpha*w[a]) (128x128 scaled identity,
    host-staged), rhs = feature-a slab (128x512). PSUM accumulates llrs in f32,
    landing dense (128, 512) j-major.
  - Min-sum reads PSUM directly (no evacuation copy): |.| on ACT, signs and
    sign-product tournament on Pool, min1/min2 tournament + leave-one-out
    select + final products on DVE, broadcast materialization on ACT.
  - Only the llr plane (vals) is written back; the host assembles the full
    output (copy of untouched input channels + channel-0 scatter), pure data
    movement.
"""

import os
import sys
from contextlib import ExitStack

import numpy as np

for _p in ("/opt/trn_rl_repo", "/opt/trn_rl_repo/concourse"):
    if _p not in sys.path and os.path.isdir(_p):
        sys.path.insert(0, _p)

# ---- problem geometry (fixed by the spec) ----
B, M, H = 4, 393216, 64
C, D = 49152, 8
NCORES = 8
CS = C // NCORES          # 6144 checks per core
CI = B * CS               # 24576 check-instances per core (batch-major)
PT = 128                  # partitions
NG = 3                    # PSUM bank-groups per core
NT = 64                   # checks per partition per group
GW = D * NT               # 512 llrs per partition per group (j-major)
NSUB = 8                  # DMA sub-tiles per group (8 feature-slabs each)
SUBW = 8 * GW             # 4096 elements per sub-tile per partition

_CACHE: dict = {}

# test-harness hooks: extra kwargs for run_bass_kernel_spmd (e.g. tracing) and
# the last BassKernelResults for reading exec_time_ns. Unused when grading.
RUN_KW: dict = {}
last_results = None


def _build(bias: float):
    """Trace + compile the per-core Bass kernel.

    Inputs:
      x: (NG, PT, H*GW) f16 -- per-core message features, bank-group-major,
         free = a*GW + j*NT + t  (feature-slab-major, j-major messages)
      w: (PT, H*PT) f16     -- 64 concatenated 128x128 scaled identities,
         w[p, a*PT + p] = alpha*proj_w[a]
    Output:
      o: (NG, PT, GW) f32   -- min-sum vals, same j-major layout as x slabs
    """
    import concourse.bass as bass  # noqa: F401
    import concourse.tile as tile
    from concourse import bacc, mybir

    f32 = mybir.dt.float32
    f16 = mybir.dt.bfloat16
    op = mybir.AluOpType

    nc = bacc.Bacc(
        "TRN2",
        target_bir_lowering=False,
        debug=False,
        enable_asserts=False,
        num_devices=NCORES,
    )
    x_d = nc.dram_tensor("x", [NG, PT, NSUB * SUBW], f16, kind="ExternalInput").ap()
    w_d = nc.dram_tensor("w", [PT, H * PT], f16, kind="ExternalInput").ap()
    o_d = nc.dram_tensor("o", [NG, PT, GW], f32, kind="ExternalOutput").ap()

    with tile.TileContext(nc) as tc, ExitStack() as ctx:
        wpool = ctx.enter_context(tc.tile_pool(name="wid", bufs=1))
        xpool = ctx.enter_context(tc.tile_pool(name="x", bufs=8))
        pspool = ctx.enter_context(tc.tile_pool(name="ps", bufs=3, space="PSUM"))
        mpool = ctx.enter_context(tc.tile_pool(name="ms", bufs=2))

        # identity-weight tile arrives in 8 chunks interleaved with group-0
        # x sub-tiles so the first matmul doesn't wait on the full 2MB
        w_t = wpool.tile([PT, H * PT], f16)
        WCH = H * PT // NSUB
        for g in range(NG):
            ps = pspool.tile([PT, GW], f32, tag="ps")
            for s in range(NSUB):
                if g == 0:
                    nc.sync.dma_start(
                        w_t[:, s * WCH : (s + 1) * WCH],
                        w_d[:, s * WCH : (s + 1) * WCH],
                    )
                xt = xpool.tile([PT, SUBW], f16, tag="xt")
                nc.sync.dma_start(xt[:], x_d[g, :, s * SUBW : (s + 1) * SUBW])
                for al in range(8):
                    a = s * 8 + al
                    nc.tensor.matmul(
                        ps[:],
                        w_t[:, a * PT : (a + 1) * PT],
                        xt[:, al * GW : (al + 1) * GW],
                        start=(s == 0 and al == 0),
                        stop=(s == NSUB - 1 and al == 7),
                    )

            # ---- leave-one-out min-sum on ps (PT, GW), j-major, nt=NT ----
            # |g| on ACT straight out of PSUM; sign as 2*(g>=0)-1 in {-1,+1}
            # on Pool. Never-zero sign keeps the leave-one-out sign product
            # correct even if an llr rounds to exactly 0.
            g_src = ps[:]
            if bias != 0.0:
                gb = mpool.tile([PT, GW], f32, tag="gb")
                nc.gpsimd.tensor_scalar_add(gb[:], ps[:], bias)
                g_src = gb[:]
            a_t = mpool.tile([PT, GW], f32, tag="abs")
            nc.scalar.activation(a_t[:], g_src, mybir.ActivationFunctionType.Abs)
            sge = mpool.tile([PT, GW], f32, tag="sge")
            s_t = mpool.tile([PT, GW], f32, tag="sgn")
            nc.vector.tensor_scalar(sge[:], g_src, 0.0, None, op0=op.is_ge)
            nc.gpsimd.tensor_scalar(s_t[:], sge[:], 2.0, -1.0, op0=op.mult,
                                    op1=op.add)

            q = GW // 2
            # min/max tournament for min1/min2 (exact 2nd order statistic)
            lo1 = mpool.tile([PT, q], f32, tag="lo1")
            hi1 = mpool.tile([PT, q], f32, tag="hi1")
            nc.vector.tensor_tensor(lo1[:], a_t[:, 0:q], a_t[:, q:GW], op=op.min)
            nc.vector.tensor_tensor(hi1[:], a_t[:, 0:q], a_t[:, q:GW], op=op.max)

            m1_2 = mpool.tile([PT, q // 2], f32, tag="m1_2")
            x2 = mpool.tile([PT, q // 2], f32, tag="x2")
            y2 = mpool.tile([PT, q // 2], f32, tag="y2")
            m2_2 = mpool.tile([PT, q // 2], f32, tag="m2_2")
            nc.vector.tensor_tensor(m1_2[:], lo1[:, 0 : q // 2], lo1[:, q // 2 : q], op=op.min)
            nc.vector.tensor_tensor(x2[:], lo1[:, 0 : q // 2], lo1[:, q // 2 : q], op=op.max)
            nc.vector.tensor_tensor(y2[:], hi1[:, 0 : q // 2], hi1[:, q // 2 : q], op=op.min)
            nc.vector.tensor_tensor(m2_2[:], x2[:], y2[:], op=op.min)

            min1 = mpool.tile([PT, NT], f32, tag="min1")
            x3 = mpool.tile([PT, NT], f32, tag="x3")
            y3 = mpool.tile([PT, NT], f32, tag="y3")
            min2 = mpool.tile([PT, NT], f32, tag="min2")
            nc.vector.tensor_tensor(min1[:], m1_2[:, 0:NT], m1_2[:, NT : 2 * NT], op=op.min)
            nc.vector.tensor_tensor(x3[:], m1_2[:, 0:NT], m1_2[:, NT : 2 * NT], op=op.max)
            nc.vector.tensor_tensor(y3[:], m2_2[:, 0:NT], m2_2[:, NT : 2 * NT], op=op.min)
            nc.vector.tensor_tensor(min2[:], x3[:], y3[:], op=op.min)

            # sign product per check (tournament of multiplies) on Pool;
            # runs concurrently with the DVE min tournament above
            s1 = mpool.tile([PT, q], f32, tag="s1")
            nc.gpsimd.tensor_tensor(s1[:], s_t[:, 0:q], s_t[:, q:GW], op=op.mult)
            s2 = mpool.tile([PT, q // 2], f32, tag="s2")
            nc.gpsimd.tensor_tensor(s2[:], s1[:, 0 : q // 2], s1[:, q // 2 : q], op=op.mult)
            ts = mpool.tile([PT, NT], f32, tag="ts")
            nc.gpsimd.tensor_tensor(ts[:], s2[:, 0:NT], s2[:, NT : 2 * NT], op=op.mult)

            # leave-one-out sign sl = s_t * bcast(ts): off the critical path
            # (ready before the min tournament finishes)
            ts_b = ts[:].unsqueeze(1).broadcast_to([PT, D, NT])
            tsf = mpool.tile([PT, GW], f32, tag="tsf")
            nc.scalar.copy(tsf[:].rearrange("p (j t) -> p j t", t=NT), ts_b)
            sl = mpool.tile([PT, GW], f32, tag="sl")
            nc.gpsimd.tensor_tensor(sl[:], s_t[:], tsf[:], op=op.mult)

            # materialize min1/min2 broadcasts on ACT and Pool in parallel
            min1_b = min1[:].unsqueeze(1).broadcast_to([PT, D, NT])
            min2_b = min2[:].unsqueeze(1).broadcast_to([PT, D, NT])
            loo = mpool.tile([PT, GW], f32, tag="loo")
            m2f = mpool.tile([PT, GW], f32, tag="m2f")
            nc.scalar.copy(loo[:].rearrange("p (j t) -> p j t", t=NT), min1_b)
            nc.gpsimd.tensor_copy(m2f[:].rearrange("p (j t) -> p j t", t=NT), min2_b)

            # loo_min = where(|g| == min1, min2, min1), then one fused final
            # product vals = loo_min * sl   (alpha already folded into w)
            msk = mpool.tile([PT, GW], mybir.dt.uint8, tag="msk")
            nc.vector.tensor_tensor(msk[:], a_t[:], loo[:], op=op.is_equal)
            nc.vector.copy_predicated(loo[:], msk[:], m2f[:])
            v2_t = mpool.tile([PT, GW], f32, tag="v2")
            nc.vector.tensor_tensor(v2_t[:], loo[:], sl[:], op=op.mult)
            # out-DMA issued from DVE (not sync) so it can never stall the
            # in-order sync DMA queue that streams the next group's x tiles
            nc.gpsimd.dma_start(o_d[g], v2_t[:])

    nc.compile()
    return nc


def _get_compiled(bias: float):
    key = (bias,)
    if key not in _CACHE:
        _CACHE[key] = _build(bias)
    return _CACHE[key]


def _prepare(message_features, proj_w, proj_b, alpha):
    """Shard/stage host-side: returns (mf, in_maps, bias)."""
    mf = np.ascontiguousarray(np.asarray(message_features, dtype=np.float32))
    w = np.asarray(proj_w, dtype=np.float32).reshape(H)
    al = float(np.asarray(alpha))
    pb = float(np.asarray(proj_b))
    assert al > 0.0, "kernel assumes alpha > 0 (scaling folded into proj_w)"

    import ml_dtypes
    bf16 = ml_dtypes.bfloat16
    wt = (w * al).astype(bf16)
    wid = np.zeros((PT, H, PT), dtype=bf16)
    wid[np.arange(PT)[:, None], :, np.arange(PT)[:, None]] = wt[None, :]
    wid = wid.reshape(PT, H * PT)
    bias = al * pb

    # per-core staging: check-instance ci = g*8192 + p*64 + t = b*6144 + c
    xr = mf.reshape(B, NCORES, CS * D * H)
    in_maps = []
    for k in range(NCORES):
        xg = xr[:, k].reshape(NG, PT, NT, D, H)         # (g, p, t, j, h)
        xk = xg.transpose(0, 1, 4, 3, 2)                # (g, p, h, j, t)
        xk = np.ascontiguousarray(xk.astype(bf16)).reshape(NG, PT, NSUB * SUBW)
        in_maps.append({"x": xk, "w": wid})
    return mf, in_maps, bias


def _assemble(mf, outs):
    """outs: per-core 'o' arrays (NG, PT, GW) f32 in j-major layout."""
    llr = np.stack(outs)                                   # (K, NG, PT, D*NT)
    llr = llr.reshape(NCORES, NG, PT, D, NT)
    llr = llr.transpose(0, 1, 2, 4, 3).reshape(NCORES, B, CS * D)  # (k, ci, j)
    llr = llr.transpose(1, 0, 2).reshape(B, M)
    out = mf.copy()
    out[:, :, 0] = llr
    return out


def kernel(
    message_features: np.ndarray,
    message_types: np.ndarray,
    check_index_tensor: np.ndarray,
    proj_w: np.ndarray,
    proj_b: np.ndarray,
    alpha: np.ndarray,
) -> np.ndarray:
    from concourse.bass_utils import run_bass_kernel_spmd

    mf, in_maps, bias = _prepare(message_features, proj_w, proj_b, alpha)
    nc = _get_compiled(bias)
    res = run_bass_kernel_spmd(nc, in_maps, core_ids=list(range(NCORES)), **RUN_KW)
    global last_results
    last_results = res
    return _assemble(mf, [r["o"] for r in res.results])
